# revision 39
# baseline (speedup 1.0000x reference)
"""MoE routed dense layer (nn_MultiHeadDense): y[b] = x[b] @ W[idx[b]] + bias[idx[b]].

Full shapes: inputs [4096,1024] f32, indices [4096] int, kernel [8,1024,1024] f32,
bias [8,1024] f32 -> out [4096,1024] f32.

Sharding strategy (expert-parallel, H == n_cores == 8): core h owns expert h's
weight [1024,1024] and processes exactly the rows routed to expert h. The host
computes the per-expert row lists from `indices`, gathers each expert's rows
into a zero-padded transposed activation block XT_h [D, C] (C = padded max
group size), and scatters the per-core outputs back into the full [B, F]
result. This does 1/8th the FLOPs of the dense all-heads reference and loads
each expert weight exactly once, on exactly one core.

On-device per core: Y[c, f] = sum_k XT[k*128:(k+1)*128, c].T @ W[k*128:.., f]
accumulated in PSUM over the 8 k-tiles, bias added during the PSUM->SBUF
eviction. X and W are pre-cast to fp16 on the host (11-bit mantissa keeps the
absmax error ~1e-3 of output scale while halving HBM traffic and enabling the
fast PE weight-load path); accumulation stays fp32 in PSUM and bias is added
in fp32.
"""

from contextlib import ExitStack

import numpy as np

import concourse.bass as bass
import concourse.tile as tile
from concourse import bacc, mybir
from concourse.bass_utils import run_bass_kernel_spmd

F32 = mybir.dt.float32
F16 = mybir.dt.float16

P = 128          # SBUF partitions / matmul tile edge
NTILE = 512      # matmul moving free dim (one fp32 PSUM bank)
WARMUP_MM = 11    # zero-matmuls bridging PE idle until chunk 0 lands


def _chunks(first, rest, total):
    out = list(first)
    while sum(out) < total:
        out.append(min(rest, total - sum(out)))
    return out


def _plan(C, D, F):
    """Shared host/device plan: k chunks, m tiles.

    W and X stream as ONE host-interleaved sequence of per-chunk blocks on
    a single HWDGE ring: chunk c is a [P, kg*(F+C)] fp16 block whose
    partition line holds, for each of its kg k-tiles, that k-tile's W row
    (F values) followed by its X row (C values). One DMA per chunk, FIFO
    on one ring: arrival order is exactly consumption order, lines are
    ~3-6 KB (the DMA engines are packet-rate-limited, so fat lines set
    the rate), and chunk completions aren't delayed by a second ring's
    packets round-robining on the same SDMA engines.
    """
    KT = D // P
    NT = F // NTILE
    kchunks = _chunks([1, 1], 2, KT)
    msizes = []
    off = 0
    while off < C:
        msizes.append(min(P, C - off))
        off += P
    moffs = list(np.cumsum([0] + msizes[:-1]))
    return KT, NT, kchunks, msizes, moffs


def _build(nc: bass.Bass, C: int, D: int, F: int):
    KT, NT, kchunks, msizes, moffs = _plan(C, D, F)
    Q = F + C        # columns per k-tile in the fused stream

    wx = nc.dram_tensor("wx", (KT * P * Q,), F16, kind="ExternalInput").ap()
    b = nc.dram_tensor("b", (P, F), F32, kind="ExternalInput").ap()
    y = nc.dram_tensor("y", (C, F), F32, kind="ExternalOutput").ap()

    with tile.TileContext(nc) as tc, ExitStack() as ctx:
        cp = ctx.enter_context(tc.tile_pool(name="cp", bufs=1))
        bp = ctx.enter_context(tc.tile_pool(name="bp", bufs=1))
        zp = ctx.enter_context(tc.tile_pool(name="zp", bufs=1))
        pp = ctx.enter_context(tc.tile_pool(name="pp", bufs=4, space="PSUM"))
        yp = ctx.enter_context(tc.tile_pool(name="yp", bufs=4))

        # The fused W+X chunks stream on the SP HWDGE ring; bias (only
        # needed at the first PSUM eviction) and the output tiles use the
        # ACT ring so they never contend with the input stream.
        wx_c = []
        off = 0
        for c, kg in enumerate(kchunks):
            q = kg * Q
            ct = cp.tile([P, q], F16, name=f"wx{c}", tag=f"wx{c}")
            nc.sync.dma_start(
                ct[:], wx[off:off + P * q].rearrange("(p q) -> p q", p=P)
            )
            wx_c.append(ct)
            off += P * q
        bias_t = bp.tile([P, F], F32)
        nc.scalar.dma_start(bias_t[:], b[:])

        # Each m-tile's PSUM is one 2-bank [P, F] tile; each matmul writes
        # one 512-column (single-bank) half. Eviction is then a single
        # [P, F] DVE add and a single 512 KB output DMA with 4 KB
        # per-partition lines (2 KB output lines were packet-rate-limited
        # to ~150 GB/s and dominated the kernel tail).
        MF = sum(1 for s in msizes if s == P)
        ps = [pp.tile([P, F], F32, name=f"ps{m}", tag="ps")
              for m in range(MF)]

        # PE warmup: zero matmuls (no DMA dependency) keep the PE busy
        # until chunk 0's completion receipt lands, so the HAM clock-gate
        # warmup (~3.5us of sustained activity before the PE runs at 2.4
        # GHz) overlaps the DMA fill instead of following it. They target
        # ps[0], which the first real k=0 matmul resets via start=True.
        zt = zp.tile([P, NTILE], F16)
        nc.vector.memset(zt[:], 0.0)
        for _ in range(WARMUP_MM):
            nc.tensor.matmul(ps[0][:, :NTILE], lhsT=zt[:, :P], rhs=zt[:],
                             start=True, stop=True)

        kmap = []  # k -> (chunk, index within chunk)
        for c, kg in enumerate(kchunks):
            kmap.extend((c, ki) for ki in range(kg))

        def mm(ps_ap, msz, moff, k, n):
            c, ki = kmap[k]
            t = wx_c[c]
            nc.tensor.matmul(
                ps_ap[:msz, n * NTILE:(n + 1) * NTILE],
                lhsT=t[:, ki * Q + F + moff:ki * Q + F + moff + msz],
                rhs=t[:, ki * Q + n * NTILE:ki * Q + (n + 1) * NTILE],
                start=(k == 0),
                stop=(k == KT - 1),
            )

        def evict(ps_ap, m, msz, moff):
            yt = yp.tile([P, F], F32, name=f"yt{m}", tag="y")
            nc.vector.tensor_add(yt[:msz, :], ps_ap[:msz, :], bias_t[:msz, :])
            nc.scalar.dma_start(y[moff:moff + msz, :], yt[:msz, :])

        # Single main pass: 4 full m-tiles x 2 n-halves over all 8 PSUM
        # banks, k outermost. The last chunk runs m-outer so each m-tile's
        # eviction starts a few matmuls before the next m-tile finishes.
        # The partial m-tile (if any) runs as its own k-block at the end:
        # its different tile_size doesn't perturb the main matmul stream,
        # and its matmuls overlap the full tiles' evictions + output DMAs,
        # leaving only its own tiny eviction as the kernel tail.
        klast = KT - kchunks[-1]
        for k in range(klast):
            for m in range(MF):
                for n in range(NT):
                    mm(ps[m], P, moffs[m], k, n)
        for m in range(MF):
            for k in range(klast, KT):
                for n in range(NT):
                    mm(ps[m], P, moffs[m], k, n)
            evict(ps[m], m, P, moffs[m])
        for m in range(MF, len(msizes)):
            msz = msizes[m]
            psr = pp.tile([P, F], F32, name=f"psr{m}", tag="ps")
            for k in range(KT):
                for n in range(NT):
                    mm(psr, msz, moffs[m], k, n)
            evict(psr, m, msz, moffs[m])


LAST_PROFILE = {}


def kernel(inputs, indices, kernel, bias, _trace=False):
    x = np.ascontiguousarray(np.asarray(inputs), dtype=np.float32)
    idx = np.asarray(indices).astype(np.int64)
    wk = np.asarray(kernel, dtype=np.float32)
    bv = np.asarray(bias, dtype=np.float32)

    B, D = x.shape
    H, _, F = wk.shape

    rows = [np.nonzero(idx == h)[0] for h in range(H)]
    maxc = max(len(r) for r in rows)
    C = max(((maxc + 15) // 16) * 16, 16)

    KT, NT, kchunks, _, _ = _plan(C, D, F)

    def pack(w16, xt16):
        # fused stream: per k-chunk one [P, kg*(F+C)] block where
        # block[p, ki*(F+C) + 0:F]   = W[(k0+ki)*P + p, :]
        # block[p, ki*(F+C) + F:F+C] = XT[(k0+ki)*P + p, :]
        KTl = w16.shape[0] // P
        fused = np.concatenate(
            [w16.reshape(KTl, P, F), xt16.reshape(KTl, P, C)], axis=2
        )  # [KT, P, F+C]
        parts = []
        k0 = 0
        for kg in kchunks:
            blk = fused[k0:k0 + kg]  # [kg, P, Q]
            parts.append(blk.transpose(1, 0, 2).reshape(-1))
            k0 += kg
        return np.concatenate(parts)

    in_maps = []
    for h in range(H):
        r = rows[h]
        xt = np.zeros((D, C), dtype=np.float16)
        xt[:, :len(r)] = x[r].T
        in_maps.append({
            "wx": pack(wk[h].astype(np.float16), xt),
            "b": np.broadcast_to(bv[h], (P, F)).copy(),
        })

    nc = bacc.Bacc(
        "TRN2", target_bir_lowering=False, debug=False, num_devices=H,
        enable_asserts=False,
    )
    _build(nc, C, D, F)
    nc.compile()

    trace_kwargs = (
        {"trace": True, "trace_cores": list(range(H)), "stitch_traces": False}
        if _trace
        else {}
    )
    res = run_bass_kernel_spmd(nc, in_maps, core_ids=list(range(H)), **trace_kwargs)
    if _trace:
        LAST_PROFILE.clear()
        LAST_PROFILE.update(
            exec_time_ns=res.exec_time_ns,
            mean_exec_time_ns=res.mean_exec_time_ns,
            max_exec_time_core_id=res.max_exec_time_core_id,
            trace=res.instructions_and_trace[1] if res.instructions_and_trace else None,
            profile_json=res.profile_json,
        )

    out = np.empty((B, F), dtype=np.float32)
    for h in range(H):
        r = rows[h]
        out[r] = res.results[h]["y"][:len(r)]
    return out


# revision 40
# speedup vs baseline: 1.0138x; 1.0138x over previous
"""MoE routed dense layer (nn_MultiHeadDense): y[b] = x[b] @ W[idx[b]] + bias[idx[b]].

Full shapes: inputs [4096,1024] f32, indices [4096] int, kernel [8,1024,1024] f32,
bias [8,1024] f32 -> out [4096,1024] f32.

Sharding strategy (expert-parallel, H == n_cores == 8): core h owns expert h's
weight [1024,1024] and processes exactly the rows routed to expert h. The host
computes the per-expert row lists from `indices`, gathers each expert's rows
into a zero-padded transposed activation block XT_h [D, C] (C = padded max
group size), and scatters the per-core outputs back into the full [B, F]
result. This does 1/8th the FLOPs of the dense all-heads reference and loads
each expert weight exactly once, on exactly one core.

On-device per core: Y[c, f] = sum_k XT[k*128:(k+1)*128, c].T @ W[k*128:.., f]
accumulated in PSUM over the 8 k-tiles, bias added during the PSUM->SBUF
eviction. X and W are pre-cast to fp16 on the host (11-bit mantissa keeps the
absmax error ~1e-3 of output scale while halving HBM traffic and enabling the
fast PE weight-load path); accumulation stays fp32 in PSUM and bias is added
in fp32.
"""

from contextlib import ExitStack

import numpy as np

import concourse.bass as bass
import concourse.tile as tile
from concourse import bacc, mybir
from concourse.bass_utils import run_bass_kernel_spmd

F32 = mybir.dt.float32
F16 = mybir.dt.float16

P = 128          # SBUF partitions / matmul tile edge
NTILE = 512      # matmul moving free dim (one fp32 PSUM bank)
WARMUP_MM = 8    # zero-matmuls bridging PE idle until chunk 0 lands


def _chunks(first, rest, total):
    out = list(first)
    while sum(out) < total:
        out.append(min(rest, total - sum(out)))
    return out


def _plan(C, D, F):
    """Shared host/device plan: k chunks, m tiles.

    W and X stream as ONE host-interleaved sequence of per-chunk blocks on
    a single HWDGE ring: chunk c is a [P, kg*(F+C)] fp16 block whose
    partition line holds, for each of its kg k-tiles, that k-tile's W row
    (F values) followed by its X row (C values). One DMA per chunk, FIFO
    on one ring: arrival order is exactly consumption order, lines are
    ~3-6 KB (the DMA engines are packet-rate-limited, so fat lines set
    the rate), and chunk completions aren't delayed by a second ring's
    packets round-robining on the same SDMA engines.
    """
    KT = D // P
    NT = F // NTILE
    kchunks = _chunks([1, 1], 2, KT)
    msizes = []
    off = 0
    while off < C:
        msizes.append(min(P, C - off))
        off += P
    moffs = list(np.cumsum([0] + msizes[:-1]))
    return KT, NT, kchunks, msizes, moffs


def _build(nc: bass.Bass, C: int, D: int, F: int):
    KT, NT, kchunks, msizes, moffs = _plan(C, D, F)
    Q = F + C        # columns per k-tile in the fused stream

    wx = nc.dram_tensor("wx", (KT * P * Q,), F16, kind="ExternalInput").ap()
    b = nc.dram_tensor("b", (P, F), F32, kind="ExternalInput").ap()
    y = nc.dram_tensor("y", (C, F), F32, kind="ExternalOutput").ap()

    with tile.TileContext(nc) as tc, ExitStack() as ctx:
        cp = ctx.enter_context(tc.tile_pool(name="cp", bufs=1))
        bp = ctx.enter_context(tc.tile_pool(name="bp", bufs=1))
        zp = ctx.enter_context(tc.tile_pool(name="zp", bufs=1))
        pp = ctx.enter_context(tc.tile_pool(name="pp", bufs=4, space="PSUM"))
        yp = ctx.enter_context(tc.tile_pool(name="yp", bufs=4))

        # The fused W+X chunks stream on the SP HWDGE ring; bias (only
        # needed at the first PSUM eviction) and the output tiles use the
        # ACT ring so they never contend with the input stream.
        wx_c = []
        off = 0
        for c, kg in enumerate(kchunks):
            q = kg * Q
            ct = cp.tile([P, q], F16, name=f"wx{c}", tag=f"wx{c}")
            nc.sync.dma_start(
                ct[:], wx[off:off + P * q].rearrange("(p q) -> p q", p=P)
            )
            wx_c.append(ct)
            off += P * q
        bias_t = bp.tile([P, F], F32)
        nc.scalar.dma_start(bias_t[:], b[:])

        # Each m-tile's PSUM is one 2-bank [P, F] tile; each matmul writes
        # one 512-column (single-bank) half. Eviction is then a single
        # [P, F] DVE add and a single 512 KB output DMA with 4 KB
        # per-partition lines (2 KB output lines were packet-rate-limited
        # to ~150 GB/s and dominated the kernel tail).
        MF = sum(1 for s in msizes if s == P)
        ps = [pp.tile([P, F], F32, name=f"ps{m}", tag="ps")
              for m in range(MF)]

        # PE warmup: zero matmuls (no DMA dependency) keep the PE busy
        # until chunk 0's completion receipt lands, so the HAM clock-gate
        # warmup (~3.5us of sustained activity before the PE runs at 2.4
        # GHz) overlaps the DMA fill instead of following it. They target
        # ps[0], which the first real k=0 matmul resets via start=True.
        zt = zp.tile([P, NTILE], F16)
        nc.vector.memset(zt[:], 0.0)
        for _ in range(WARMUP_MM):
            nc.tensor.matmul(ps[0][:, :NTILE], lhsT=zt[:, :P], rhs=zt[:],
                             start=True, stop=True)

        kmap = []  # k -> (chunk, index within chunk)
        for c, kg in enumerate(kchunks):
            kmap.extend((c, ki) for ki in range(kg))

        def mm(ps_ap, msz, moff, k, n):
            c, ki = kmap[k]
            t = wx_c[c]
            nc.tensor.matmul(
                ps_ap[:msz, n * NTILE:(n + 1) * NTILE],
                lhsT=t[:, ki * Q + F + moff:ki * Q + F + moff + msz],
                rhs=t[:, ki * Q + n * NTILE:ki * Q + (n + 1) * NTILE],
                start=(k == 0),
                stop=(k == KT - 1),
            )

        def evict(ps_ap, m, msz, moff):
            yt = yp.tile([P, F], F32, name=f"yt{m}", tag="y")
            nc.vector.tensor_add(yt[:msz, :], ps_ap[:msz, :], bias_t[:msz, :])
            nc.scalar.dma_start(y[moff:moff + msz, :], yt[:msz, :])

        # Single main pass: 4 full m-tiles x 2 n-halves over all 8 PSUM
        # banks, k outermost. The last chunk runs m-outer so each m-tile's
        # eviction starts a few matmuls before the next m-tile finishes.
        # The partial m-tile (if any) runs as its own k-block at the end:
        # its different tile_size doesn't perturb the main matmul stream,
        # and its matmuls overlap the full tiles' evictions + output DMAs,
        # leaving only its own tiny eviction as the kernel tail.
        klast = KT - kchunks[-1]
        for k in range(klast):
            for m in range(MF):
                for n in range(NT):
                    mm(ps[m], P, moffs[m], k, n)
        for m in range(MF):
            for k in range(klast, KT):
                for n in range(NT):
                    mm(ps[m], P, moffs[m], k, n)
            evict(ps[m], m, P, moffs[m])
        for m in range(MF, len(msizes)):
            msz = msizes[m]
            psr = pp.tile([P, F], F32, name=f"psr{m}", tag="ps")
            for k in range(KT):
                for n in range(NT):
                    mm(psr, msz, moffs[m], k, n)
            evict(psr, m, msz, moffs[m])


LAST_PROFILE = {}


def kernel(inputs, indices, kernel, bias, _trace=False):
    x = np.ascontiguousarray(np.asarray(inputs), dtype=np.float32)
    idx = np.asarray(indices).astype(np.int64)
    wk = np.asarray(kernel, dtype=np.float32)
    bv = np.asarray(bias, dtype=np.float32)

    B, D = x.shape
    H, _, F = wk.shape

    rows = [np.nonzero(idx == h)[0] for h in range(H)]
    maxc = max(len(r) for r in rows)
    C = max(((maxc + 15) // 16) * 16, 16)

    KT, NT, kchunks, _, _ = _plan(C, D, F)

    def pack(w16, xt16):
        # fused stream: per k-chunk one [P, kg*(F+C)] block where
        # block[p, ki*(F+C) + 0:F]   = W[(k0+ki)*P + p, :]
        # block[p, ki*(F+C) + F:F+C] = XT[(k0+ki)*P + p, :]
        KTl = w16.shape[0] // P
        fused = np.concatenate(
            [w16.reshape(KTl, P, F), xt16.reshape(KTl, P, C)], axis=2
        )  # [KT, P, F+C]
        parts = []
        k0 = 0
        for kg in kchunks:
            blk = fused[k0:k0 + kg]  # [kg, P, Q]
            parts.append(blk.transpose(1, 0, 2).reshape(-1))
            k0 += kg
        return np.concatenate(parts)

    in_maps = []
    for h in range(H):
        r = rows[h]
        xt = np.zeros((D, C), dtype=np.float16)
        xt[:, :len(r)] = x[r].T
        in_maps.append({
            "wx": pack(wk[h].astype(np.float16), xt),
            "b": np.broadcast_to(bv[h], (P, F)).copy(),
        })

    nc = bacc.Bacc(
        "TRN2", target_bir_lowering=False, debug=False, num_devices=H,
        enable_asserts=False,
    )
    _build(nc, C, D, F)
    nc.compile()

    trace_kwargs = (
        {"trace": True, "trace_cores": list(range(H)), "stitch_traces": False}
        if _trace
        else {}
    )
    res = run_bass_kernel_spmd(nc, in_maps, core_ids=list(range(H)), **trace_kwargs)
    if _trace:
        LAST_PROFILE.clear()
        LAST_PROFILE.update(
            exec_time_ns=res.exec_time_ns,
            mean_exec_time_ns=res.mean_exec_time_ns,
            max_exec_time_core_id=res.max_exec_time_core_id,
            trace=res.instructions_and_trace[1] if res.instructions_and_trace else None,
            profile_json=res.profile_json,
        )

    out = np.empty((B, F), dtype=np.float32)
    for h in range(H):
        r = rows[h]
        out[r] = res.results[h]["y"][:len(r)]
    return out


# revision 45
# speedup vs baseline: 1.1968x; 1.1805x over previous
"""MoE routed dense layer (nn_MultiHeadDense): y[b] = x[b] @ W[idx[b]] + bias[idx[b]].

Full shapes: inputs [4096,1024] f32, indices [4096] int, kernel [8,1024,1024] f32,
bias [8,1024] f32 -> out [4096,1024] f32.

Sharding strategy (expert-parallel, H == n_cores == 8): core h owns expert h's
weight [1024,1024] and processes exactly the rows routed to expert h. The host
computes the per-expert row lists from `indices`, gathers each expert's rows
into a zero-padded transposed activation block XT_h [D, C] (C = padded max
group size), and scatters the per-core outputs back into the full [B, F]
result. This does 1/8th the FLOPs of the dense all-heads reference and loads
each expert weight exactly once, on exactly one core.

On-device per core: Y[c, f] = sum_k XT[k*128:(k+1)*128, c].T @ W[k*128:.., f]
accumulated in PSUM over the 8 k-tiles, bias added during the PSUM->SBUF
eviction. X and W are pre-cast to fp16 on the host (11-bit mantissa keeps the
absmax error ~1e-3 of output scale while halving HBM traffic and enabling the
fast PE weight-load path); accumulation stays fp32 in PSUM and bias is added
in fp32.
"""

from contextlib import ExitStack

import numpy as np

import concourse.bass as bass
import concourse.tile as tile
from concourse import bacc, mybir
from concourse.bass_utils import run_bass_kernel_spmd

F32 = mybir.dt.float32
F16 = mybir.dt.float16

P = 128          # SBUF partitions / matmul tile edge
NTILE = 512      # matmul moving free dim (one fp32 PSUM bank)
WARMUP_MM = 8    # zero-matmuls bridging PE idle until chunk 0 lands


def _chunks(first, rest, total):
    out = list(first)
    while sum(out) < total:
        out.append(min(rest, total - sum(out)))
    return out


def _plan(C, D, F):
    """Shared host/device plan: k chunks, m tiles.

    W and X stream as ONE host-interleaved sequence of per-chunk blocks on
    a single HWDGE ring: chunk c is a [P, kg*(F+C)] fp16 block whose
    partition line holds, for each of its kg k-tiles, that k-tile's W row
    (F values) followed by its X row (C values). One DMA per chunk, FIFO
    on one ring: arrival order is exactly consumption order, lines are
    ~3-6 KB (the DMA engines are packet-rate-limited, so fat lines set
    the rate), and chunk completions aren't delayed by a second ring's
    packets round-robining on the same SDMA engines.
    """
    KT = D // P
    NT = F // NTILE
    kchunks = _chunks([1, 1], 2, KT)
    msizes = []
    off = 0
    while off < C:
        msizes.append(min(P, C - off))
        off += P
    moffs = list(np.cumsum([0] + msizes[:-1]))
    return KT, NT, kchunks, msizes, moffs


def _build(nc: bass.Bass, C: int, D: int, F: int):
    KT, NT, kchunks, msizes, moffs = _plan(C, D, F)
    Q = F + C        # columns per k-tile in the fused stream

    wx = nc.dram_tensor("wx", (KT * P * Q,), F16, kind="ExternalInput").ap()
    b = nc.dram_tensor("b", (P, F), F32, kind="ExternalInput").ap()
    y = nc.dram_tensor("y", (C, F), F32, kind="ExternalOutput").ap()

    with tile.TileContext(nc) as tc, ExitStack() as ctx:
        cp = ctx.enter_context(tc.tile_pool(name="cp", bufs=1))
        bp = ctx.enter_context(tc.tile_pool(name="bp", bufs=1))
        zp = ctx.enter_context(tc.tile_pool(name="zp", bufs=1))
        pp = ctx.enter_context(tc.tile_pool(name="pp", bufs=4, space="PSUM"))
        yp = ctx.enter_context(tc.tile_pool(name="yp", bufs=5))

        # The fused W+X chunks stream on the SP HWDGE ring; bias (only
        # needed at the first PSUM eviction) and the output tiles use the
        # ACT ring so they never contend with the input stream.
        wx_c = []
        off = 0
        for c, kg in enumerate(kchunks):
            q = kg * Q
            ct = cp.tile([P, q], F16, name=f"wx{c}", tag=f"wx{c}")
            nc.sync.dma_start(
                ct[:], wx[off:off + P * q].rearrange("(p q) -> p q", p=P)
            )
            wx_c.append(ct)
            off += P * q
        bias_t = bp.tile([P, F], F32)
        nc.scalar.dma_start(bias_t[:], b[:])

        # Each m-tile's PSUM is one 2-bank [P, F] tile; each matmul writes
        # one 512-column (single-bank) half. Eviction is then a single
        # [P, F] DVE add and a single 512 KB output DMA with 4 KB
        # per-partition lines (2 KB output lines were packet-rate-limited
        # to ~150 GB/s and dominated the kernel tail).
        MF = sum(1 for s in msizes if s == P)
        ps0 = [pp.tile([P, F], F32, name=f"ps{m}", tag="ps")
               for m in range(min(MF, 4))]

        # PE warmup: zero matmuls (no DMA dependency) keep the PE busy
        # until chunk 0's completion receipt lands, so the HAM clock-gate
        # warmup (~3.5us of sustained activity before the PE runs at 2.4
        # GHz) overlaps the DMA fill instead of following it. They target
        # ps[0], which the first real k=0 matmul resets via start=True.
        zt = zp.tile([P, NTILE], F16)
        nc.vector.memset(zt[:], 0.0)
        for _ in range(WARMUP_MM):
            nc.tensor.matmul(ps0[0][:, :NTILE], lhsT=zt[:, :P], rhs=zt[:],
                             start=True, stop=True)

        kmap = []  # k -> (chunk, index within chunk)
        for c, kg in enumerate(kchunks):
            kmap.extend((c, ki) for ki in range(kg))

        def mm(ps_ap, msz, moff, k, n):
            c, ki = kmap[k]
            t = wx_c[c]
            nc.tensor.matmul(
                ps_ap[:msz, n * NTILE:(n + 1) * NTILE],
                lhsT=t[:, ki * Q + F + moff:ki * Q + F + moff + msz],
                rhs=t[:, ki * Q + n * NTILE:ki * Q + (n + 1) * NTILE],
                start=(k == 0),
                stop=(k == KT - 1),
            )

        def evict(ps_ap, m, msz, moff):
            yt = yp.tile([P, F], F32, name=f"yt{m}", tag="y")
            nc.vector.tensor_add(yt[:msz, :], ps_ap[:msz, :], bias_t[:msz, :])
            nc.scalar.dma_start(y[moff:moff + msz, :], yt[:msz, :])

        # Main pass in groups of <=4 full m-tiles (4 x 2 banks = all of
        # PSUM), k outermost within a group. The last chunk of a group
        # runs m-outer so each m-tile's eviction starts a few matmuls
        # before the next m-tile finishes. The partial m-tile (if any)
        # runs as its own k-block at the end: its different tile_size
        # doesn't perturb the main matmul stream, and its matmuls overlap
        # the full tiles' evictions + output DMAs, leaving only its own
        # tiny eviction as the kernel tail.
        klast = KT - kchunks[-1]
        for g0 in range(0, MF, 4):
            gm = range(g0, min(g0 + 4, MF))
            gps = {
                m: ps0[m] if g0 == 0
                else pp.tile([P, F], F32, name=f"ps{m}", tag="ps")
                for m in gm
            }
            for k in range(klast):
                for m in gm:
                    for n in range(NT):
                        mm(gps[m], P, moffs[m], k, n)
            for m in gm:
                for k in range(klast, KT):
                    for n in range(NT):
                        mm(gps[m], P, moffs[m], k, n)
                evict(gps[m], m, P, moffs[m])
        for m in range(MF, len(msizes)):
            msz = msizes[m]
            psr = pp.tile([P, F], F32, name=f"psr{m}", tag="ps")
            for k in range(KT):
                for n in range(NT):
                    mm(psr, msz, moffs[m], k, n)
            evict(psr, m, msz, moffs[m])


LAST_PROFILE = {}


def kernel(inputs, indices, kernel, bias, _trace=False):
    x = np.ascontiguousarray(np.asarray(inputs), dtype=np.float32)
    idx = np.asarray(indices).astype(np.int64)
    wk = np.asarray(kernel, dtype=np.float32)
    bv = np.asarray(bias, dtype=np.float32)

    B, D = x.shape
    H, _, F = wk.shape

    rows = [np.nonzero(idx == h)[0] for h in range(H)]
    maxc = max(len(r) for r in rows)
    C = max(((maxc + 15) // 16) * 16, 16)

    KT, NT, kchunks, _, _ = _plan(C, D, F)

    def pack(w16, xt16):
        # fused stream: per k-chunk one [P, kg*(F+C)] block where
        # block[p, ki*(F+C) + 0:F]   = W[(k0+ki)*P + p, :]
        # block[p, ki*(F+C) + F:F+C] = XT[(k0+ki)*P + p, :]
        KTl = w16.shape[0] // P
        fused = np.concatenate(
            [w16.reshape(KTl, P, F), xt16.reshape(KTl, P, C)], axis=2
        )  # [KT, P, F+C]
        parts = []
        k0 = 0
        for kg in kchunks:
            blk = fused[k0:k0 + kg]  # [kg, P, Q]
            parts.append(blk.transpose(1, 0, 2).reshape(-1))
            k0 += kg
        return np.concatenate(parts)

    in_maps = []
    for h in range(H):
        r = rows[h]
        xt = np.zeros((D, C), dtype=np.float16)
        xt[:, :len(r)] = x[r].T
        in_maps.append({
            "wx": pack(wk[h].astype(np.float16), xt),
            "b": np.broadcast_to(bv[h], (P, F)).copy(),
        })

    nc = bacc.Bacc(
        "TRN2", target_bir_lowering=False, debug=False, num_devices=H,
        enable_asserts=False,
    )
    _build(nc, C, D, F)
    nc.compile()

    trace_kwargs = (
        {"trace": True, "trace_cores": list(range(H)), "stitch_traces": False}
        if _trace
        else {}
    )
    res = run_bass_kernel_spmd(nc, in_maps, core_ids=list(range(H)), **trace_kwargs)
    if _trace:
        LAST_PROFILE.clear()
        LAST_PROFILE.update(
            exec_time_ns=res.exec_time_ns,
            mean_exec_time_ns=res.mean_exec_time_ns,
            max_exec_time_core_id=res.max_exec_time_core_id,
            trace=res.instructions_and_trace[1] if res.instructions_and_trace else None,
            profile_json=res.profile_json,
        )

    out = np.empty((B, F), dtype=np.float32)
    for h in range(H):
        r = rows[h]
        out[r] = res.results[h]["y"][:len(r)]
    return out
